# revision 132
# baseline (speedup 1.0000x reference)
"""MultiHeadInfiniAttention Trainium2 kernel (8 NeuronCores).

Problem: B=2, T=4096, D=1024, H=8 heads x 128 dh, SEG_LEN=512 (8 segments).
Per (b,h): segment-recurrent memory (M [128,129 incl z]) + local causal
softmax attention, gated combine.

Sharding: 16 (b,h) pairs over 8 cores -> core c handles b=c//4 and heads
{2*(c%4), 2*(c%4)+1}.  Host passes per-core inputs: xT=x[b].T in bf16,
bf16 weight column slices, bias/gate columns, small constant matrices.

On-device dataflow per segment s (both heads stage-interleaved in produce):
  - qT/kT [dh,512] bf16 MMs; v projected DIRECTLY in natural [t,dh] layout
    (four token-tile regions sharing one psum bank); PE warmed up with dummy
    matmuls during the startup DMA so real MMs start at full clock
  - elu+1 = min(exp(x),1) + relu(x): exp on ACT straight off the projection
    psum (before the cast), min + max-add stt on DVE
  - scoresT_j = k_j^T q (block-causal skip); ACT exp writes P^T; Pool
    zeroes the diagonal block with a triu keep-mask (consumed LAST by the
    a_dot accumulation); a_dot accumulates against v||1 so column dh holds
    the softmax denominator
  - delta rule M||z += sk^T (v||1) + sk^T (retr * -1/(sk.z)); reciprocal
    straight off the psum z-column, negation folded into the scale stt
  - combine on DVE (per-region at the tail, stores split across DMA queues)
"""

import os
import sys

sys.path.insert(0, os.path.dirname(os.path.abspath(__file__)))

import numpy as np
import ml_dtypes

import concourse.bass as bass
import concourse.mybir as mybir
import concourse.tile as tile
from concourse import bass_utils
from concourse.bass import ts


def split_multi_waits(nc, max_waits: int = 1) -> int:
    """This container's walrus build only supports ONE sync wait per
    instruction.  Tile emits multi-wait instructions; split the extras onto
    same-engine NOP carriers inserted right before each instruction."""
    n_split = 0
    for func in nc.m.functions:
        for bb in func.blocks:
            insts = bb.instructions
            new_list = []
            changed = False
            for inst in insts:
                si = inst.sync_info
                if si is not None and si.on_wait and len(si.on_wait) > max_waits:
                    waits = list(si.on_wait)
                    for w in waits[max_waits:]:
                        nop = mybir.InstNoOp(name=f"WSPLIT-{nc.next_id()}")
                        nop.engine = inst.engine
                        nop.sync_info = mybir.SyncInfo(on_wait=[w], on_update=[])
                        new_list.append(nop)
                        n_split += 1
                    inst.sync_info = mybir.SyncInfo(
                        on_wait=waits[:max_waits],
                        on_update=list(si.on_update or []),
                    )
                    changed = True
                new_list.append(inst)
            if changed:
                bb.instructions = new_list
    return n_split


F32 = mybir.dt.float32
F32R = mybir.dt.float32r
BF16 = mybir.dt.bfloat16
AF = mybir.ActivationFunctionType
ALU = mybir.AluOpType

B, T, D = 2, 4096, 1024
H, DH, SEG = 8, 128, 512
S = T // SEG          # 8 segments
NCH = D // 128        # 8 contraction chunks
EPS = 1e-6
INV_SQRT_D = 1.0 / float(np.sqrt(DH))
MASK_NEG = -1.0e9

LAST_RESULTS = None  # BassKernelResults of the last run (for test.py)


def _build_program(has_bias=False):
    nc = bass.Bass("TRN2", target_bir_lowering=False, debug=False)

    xT = nc.dram_tensor("xT", (D, T), BF16, kind="ExternalInput")
    wq = nc.dram_tensor("wq", (D, 2 * DH), BF16, kind="ExternalInput")
    wk = nc.dram_tensor("wk", (D, 2 * DH), BF16, kind="ExternalInput")
    wv = nc.dram_tensor("wv", (D, 2 * DH), BF16, kind="ExternalInput")
    biases = nc.dram_tensor("biases", (128, 6), F32, kind="ExternalInput")
    bvT_d = nc.dram_tensor("bvT", (1, 2 * DH), BF16, kind="ExternalInput")
    gates = nc.dram_tensor("gates", (128, 4), F32, kind="ExternalInput")
    ident_d = nc.dram_tensor("ident", (128, 128), BF16, kind="ExternalInput")
    maskl_d = nc.dram_tensor("maskl", (128, 128), BF16, kind="ExternalInput")
    maskr_d = nc.dram_tensor("maskr", (128, 128), BF16, kind="ExternalInput")
    y = nc.dram_tensor("out", (T, 2 * DH), F32, kind="ExternalOutput")

    with tile.TileContext(nc) as tc:
        _emit(nc, tc, xT, wq, wk, wv, biases, gates, ident_d, maskl_d, maskr_d, y,
              bvT_d if has_bias else None)

    split_multi_waits(nc)
    return nc


def _emit(nc, tc, xT, wq, wk, wv, biases, gates, ident_d, maskl_d, maskr_d, y,
          bvT_d=None):
    from contextlib import ExitStack

    ctx = ExitStack()
    with ctx:
        singles = ctx.enter_context(tc.tile_pool(name="singles", bufs=1))
        state = ctx.enter_context(tc.tile_pool(name="state", bufs=2))
        xpool = ctx.enter_context(tc.tile_pool(name="xts", bufs=4))
        work = ctx.enter_context(tc.tile_pool(name="work", bufs=4))
        small = ctx.enter_context(tc.tile_pool(name="small", bufs=12))
        outp = ctx.enter_context(tc.tile_pool(name="outp", bufs=4))
        # PSUM pools -- exactly 8 banks
        proj_ps = ctx.enter_context(tc.tile_pool(name="proj_ps", bufs=2, space="PSUM"))
        trp_ps = proj_ps  # transposes share the projection psum slots
        sc_ps_p = ctx.enter_context(tc.tile_pool(name="sc_ps", bufs=2, space="PSUM"))
        adot_ps_p = ctx.enter_context(tc.tile_pool(name="adot_ps", bufs=2, space="PSUM"))
        mem_ps_p = ctx.enter_context(tc.tile_pool(name="mem_ps", bufs=2, space="PSUM"))

        # ---- constants ----
        # Small consts + weights go on the ACT HWDGE queue; xts slabs and
        # output stores use the SP queue, so startup overlaps.  Weights are
        # split per contraction chunk so the first projection matmuls can
        # start after ~1 chunk of wq instead of all 3 weight matrices.
        # load order on the sync queue: wq -> segment-0 x slab (split per
        # chunk for incremental matmul start) -> wk/wv -> remaining slabs
        # (one large DMA each; per-dma_start issue overhead is ~0.6us).
        w_sb = {}
        w_views = {}
        for name, dram in (("wq", wq), ("wk", wk), ("wv", wv)):
            w_sb[name] = singles.tile(
                [128, NCH, 2 * DH], BF16, tag=f"w_{name}", name=f"w_{name}"
            )
            w_views[name] = dram.ap().rearrange("(c p) n -> p c n", p=128)

        # ---- persistent per-head state ----
        # mzb double-buffered per head: segment s reads buf[(s-1)%2] (old M)
        # while the update writes buf[s%2], so the chain write never waits
        # on this segment's readers.
        mz_f32, mz_bf = [], []
        for hi in range(2):
            mzf = state.tile([128, DH + 1], F32, tag="mz_f32")
            bufs2 = [
                state.tile([128, DH + 1], BF16, tag="mz_bf", bufs=4,
                           name=f"mzb_{hi}_{k}")
                for k in range(2)
            ]
            mz_f32.append(mzf)
            mz_bf.append(bufs2)

        yv = y.ap().rearrange(
            "(s tile p) (h e) -> s p tile h e", p=128, tile=4, h=2
        )
        # x^T slab view: slab[p, c, f] = xT[c*128 + p, s*512 + f]
        xv = xT.ap().rearrange("(c p) t -> p c t", p=128)

        def load_slab(s, split):
            slab = xpool.tile([128, NCH, SEG], BF16, tag="slab", name=f"slab{s}")
            if split:
                for c in range(NCH):
                    nc.sync.dma_start(out=slab[:, c, :], in_=xv[:, c, ts(s, SEG)])
            else:
                nc.sync.dma_start(out=slab[:], in_=xv[:, :, ts(s, SEG)])
            return slab

        # PE warmup: the tensor engine runs at half clock until ~3us of
        # sustained activity.  The first real matmul can't start until the
        # wq+slab0 DMAs land (~6us), so spin no-dep dummy matmuls on a
        # zeroed tile meanwhile -- the real projections then start warm.
        warm_sb = singles.tile([128, 128], BF16, tag="warm_sb")
        nc.gpsimd.memset(warm_sb[:], 0.0)
        warm_ps = proj_ps.tile([128, SEG], F32, tag="proj", name="warm_ps")
        for i in range(47):
            nc.tensor.matmul(
                warm_ps[:, 0:128], warm_sb[:], warm_sb[:],
                start=(i == 0), stop=(i == 46), skip_group_check=True,
            )

        # Startup: DMA issue costs ~565ns SEQ time each, so use FEW large
        # transfers, ordered so the first q-projection chunks unblock ASAP.
        # Weights ride the scalar queue (overlapped issue with the SP slabs).
        slab0 = xpool.tile([128, NCH, SEG], BF16, tag="slab", name="slab0")
        nc.sync.dma_start(out=w_sb["wq"][:, 0:2, :], in_=w_views["wq"][:, 0:2, :])
        nc.sync.dma_start(out=slab0[:, 0:4, :], in_=xv[:, 0:4, ts(0, SEG)])
        nc.sync.dma_start(out=w_sb["wq"][:, 2:8, :], in_=w_views["wq"][:, 2:8, :])
        nc.sync.dma_start(out=slab0[:, 4:8, :], in_=xv[:, 4:8, ts(0, SEG)])
        nc.sync.dma_start(out=w_sb["wk"][:], in_=w_views["wk"][:])
        nc.sync.dma_start(out=w_sb["wv"][:], in_=w_views["wv"][:])
        bias_sb = singles.tile([128, 6], F32, tag="bias")
        nc.scalar.dma_start(out=bias_sb[:], in_=biases.ap())
        ident = singles.tile([128, 128], BF16, tag="ident")
        nc.scalar.dma_start(out=ident[:], in_=ident_d.ap())
        gate_sb = singles.tile([128, 4], F32, tag="gate")
        nc.scalar.dma_start(out=gate_sb[:], in_=gates.ap())
        maskl = singles.tile([128, 128], BF16, tag="maskl")
        nc.scalar.dma_start(out=maskl[:], in_=maskl_d.ap())
        maskr = singles.tile([128, 128], BF16, tag="maskr")
        nc.scalar.dma_start(out=maskr[:], in_=maskr_d.ap())

        bias_v = None
        if bvT_d is not None:
            ones_row = singles.tile([1, SEG], BF16, tag="ones_row")
            nc.gpsimd.memset(ones_row[:], 1.0)
            bvT_sb = singles.tile([1, 2 * DH], BF16, tag="bvT")
            nc.scalar.dma_start(out=bvT_sb[:], in_=bvT_d.ap())
            bias_v = (ones_row, bvT_sb)

        # Software-pipelined emission: the "produce" phase (projections, elu,
        # layout transposes) of segment s+1 is emitted before the serial
        # "scan" phase of segment s, so the scheduler can fill the scan's
        # dependency stalls with projection matmuls.
        def produce(s, slab):
            xts = [slab[:, c, :] for c in range(NCH)]
            return _produce_phase(
                nc, tc, s, xts, w_sb, bias_sb, ident,
                work, proj_ps, trp_ps, bias_v,
            )

        zstate = [None, None]
        for s in range(S):
            slab = slab0 if s == 0 else load_slab(s, split=(s == 1))
            pr = produce(s, slab)
            # layout [p, tile, head, e] so the store DMA collapses to 2D
            a2_sb = outp.tile([128, 4, 2, 128], F32, tag="a2_sb", name=f"a2_{s}")
            for hi in range(2):
                zstate[hi] = _scan_phase(
                    nc, tc, s, hi, pr[hi], gate_sb, ident, maskl, maskr,
                    mz_f32[hi], mz_bf[hi][(s - 1) % 2], mz_bf[hi][s % 2],
                    work, small,
                    sc_ps_p, trp_ps, adot_ps_p, mem_ps_p,
                    a2_sb[:, :, hi, :],
                    # last segment: store per combine-pair, each on its own
                    # queue so the issue latencies overlap
                    (lambda pair, hi=hi, s=s, a2_sb=a2_sb: [
                        nc.sync, nc.scalar, nc.scalar, nc.sync
                    ][2 * hi + pair].dma_start(
                        out=yv[s, :, 2 * pair : 2 * pair + 2, hi],
                        in_=a2_sb[:, 2 * pair : 2 * pair + 2, hi, :],
                    )) if s == S - 1 else None,
                    zprev=zstate[hi],
                )
            if s < S - 1:
                nc.sync.dma_start(out=yv[s], in_=a2_sb[:])


def _produce_phase(
    nc, tc, s, xts, w_sb, bias_sb, ident, work, proj_ps, trp_ps, bias_v,
):
    """Produce q/k/v (+elu transforms) for BOTH heads, stage-interleaved so
    each psum slot's WAR release has a full stage of slack."""
    pr = [dict() for _ in range(2)]

    # ---------- projections: qT/kT [dh, 512] ----------
    def project(wname, hi):
        ps = proj_ps.tile([128, SEG], F32, tag="proj", name=f"proj_{wname}_{s}_{hi}")
        w = w_sb[wname]
        for c in range(NCH):
            nc.tensor.matmul(
                ps[:], w[:, c, ts(hi, DH)], xts[c],
                start=(c == 0), stop=(c == NCH - 1),
            )
        return ps

    # the elu exp is emitted BEFORE the cast: the elu chain (exp->min->stt)
    # gates the next segment's retr/amem Ldweights, while the cast's readers
    # (scores) have plenty of other PE work queued ahead of them.
    def exp_part(src_ps, bias_ap, tag, hi):
        ex = work.tile([128, SEG], BF16, tag=f"ex_{tag}", bufs=4,
                       name=f"ex_{tag}_{s}_{hi}")
        nc.scalar.activation(ex[:], src_ps[:], AF.Exp, bias=bias_ap)
        return ex

    # last segment: its few produce ACT ops (casts/exps) should win the
    # in-order ACT stream over segment S-2's remaining softmax exps the
    # moment they are ready -- the tail is ACT-serialized
    from contextlib import nullcontext
    prio = tc.high_priority() if s == S - 1 else nullcontext()
    qt = [None, None]
    for hi in range(2):
        qt[hi] = project("wq", hi)
        # last segment: the wall-critical path is scores->adot->combine->
        # store, so the cast comes FIRST there; elsewhere the elu chain is
        # the priority and the exp leads
        if s == S - 1:
            with tc.high_priority():
                q_bf = work.tile([128, SEG], BF16, tag="q_bf", bufs=5,
                                 name=f"q_bf_{s}_{hi}")
                nc.scalar.activation(q_bf[:], qt[hi][:], AF.Identity,
                                     bias=bias_sb[:, 0 + hi : 1 + hi])
                pr[hi]["q_ex"] = exp_part(qt[hi], bias_sb[:, 0 + hi : 1 + hi],
                                          "q", hi)
        else:
            pr[hi]["q_ex"] = (exp_part(qt[hi], bias_sb[:, 0 + hi : 1 + hi],
                                       "q", hi) if s > 0 else None)
            q_bf = work.tile([128, SEG], BF16, tag="q_bf", bufs=5,
                             name=f"q_bf_{s}_{hi}")
            nc.scalar.activation(q_bf[:], qt[hi][:], AF.Identity,
                                 bias=bias_sb[:, 0 + hi : 1 + hi])
        pr[hi]["q_bf"] = q_bf
    kt = [None, None]
    for hi in range(2):
        kt[hi] = project("wk", hi)
        pr[hi]["k_ex"] = (exp_part(kt[hi], bias_sb[:, 2 + hi : 3 + hi], "k", hi)
                          if s < S - 1 else None)
        with (tc.high_priority() if s == S - 1 else nullcontext()):
            k_bf = work.tile([128, SEG], BF16, tag="k_bf", bufs=5,
                             name=f"k_bf_{s}_{hi}")
            nc.scalar.activation(k_bf[:], kt[hi][:], AF.Identity,
                                 bias=bias_sb[:, 2 + hi : 3 + hi])
        pr[hi]["k_bf"] = k_bf

    # ---------- v projected DIRECTLY in natural layout [t, dh] ----------
    # lhsT = x chunk (x^T is already [d, t]), rhs = wv chunk: out[t,e] =
    # sum_d x[d,t] wv[d,e].  Four token-tile regions share ONE psum bank;
    # only the first-emitted matmul carries start=True (clears the bank's
    # has_written), the other regions' first writes store via cleared bits.
    for hi in range(2):
        vnat_ps = proj_ps.tile([128, 4, DH], F32, tag="proj",
                               name=f"vnat_{s}_{hi}")
        for c in range(NCH):
            for i in range(4):
                nc.tensor.matmul(
                    vnat_ps[:, i, :], xts[c][:, ts(i, 128)],
                    w_sb["wv"][:, c, ts(hi, DH)],
                    start=(c == 0 and i == 0),
                    stop=(c == NCH - 1 and bias_v is None),
                    skip_group_check=True,
                )
        if bias_v is not None:
            ones_row, bvT_sb = bias_v
            # bias contribution ones[t] (x) b_v via a rank-1 matmul per tile
            for i in range(4):
                nc.tensor.matmul(
                    vnat_ps[:, i, :], ones_row[:, ts(i, 128)],
                    bvT_sb[:, ts(hi, DH)],
                    start=False, stop=True, skip_group_check=True,
                )
        # v_ones [m, 4, dh+1]: natural-layout v with a ones column, so the
        # a_dot matmul accumulates the softmax denominator in column dh.
        v_ones = work.tile([128, 4, DH + 1], BF16, tag="nat_v", bufs=5,
                           name=f"nat_v_{s}_{hi}")
        nc.gpsimd.memset(v_ones[:, :, DH : DH + 1], 1.0)
        nc.scalar.copy(v_ones[:, :, :DH], vnat_ps[:])
        pr[hi]["v_ones"] = v_ones

    # ---------- elu(x)+1 = min(exp(x),1) + relu(x), bf16 ----------
    def elu1(ex, src_bf, tag, hi):
        nc.vector.tensor_scalar_min(ex[:], ex[:], 1.0)
        out = work.tile([128, SEG], BF16, tag=f"s_{tag}", bufs=5,
                        name=f"s_{tag}_{s}_{hi}")
        nc.vector.scalar_tensor_tensor(
            out=out[:], in0=src_bf[:], scalar=0.0, in1=ex[:],
            op0=ALU.max, op1=ALU.add,
        )
        return out

    for hi in range(2):
        pr[hi]["sk_bf"] = (elu1(pr[hi]["k_ex"], pr[hi]["k_bf"], "k", hi)
                           if s < S - 1 else None)
        pr[hi]["sq_bf"] = (elu1(pr[hi]["q_ex"], pr[hi]["q_bf"], "q", hi)
                           if s > 0 else None)

    # ---------- natural-layout sk via PE transpose ----------
    for hi in range(2):
        sk_nat = None
        if s < S - 1:
            ps = trp_ps.tile([128, 4, 128], BF16, tag="proj",
                             name=f"trp_sk_{s}_{hi}")
            for i in range(4):
                nc.tensor.transpose(ps[:, i, :], pr[hi]["sk_bf"][:, ts(i, 128)],
                                    ident[:])
            sk_nat = work.tile([128, 4, DH], BF16, tag="nat_sk", bufs=5,
                               name=f"nat_sk_{s}_{hi}")
            nc.vector.tensor_copy(sk_nat[:], ps[:])
        pr[hi]["sk_nat"] = sk_nat

    return pr


def _scan_phase(
    nc, tc, s, hi, pr, gate_sb, ident, maskl, maskr,
    mzf, mzb_prev, mzb_new, work, small,
    sc_ps_p, trp_ps, adot_ps_p, mem_ps_p, a_sb, store_cb=None, zprev=None,
):
    q_bf, k_bf = pr["q_bf"], pr["k_bf"]
    sq_bf, sk_bf = pr["sq_bf"], pr["sk_bf"]
    v_ones, sk_nat = pr["v_ones"], pr["sk_nat"]

    # ---------- memory state pipeline ----------
    # M update is decomposed as  M||z += sk^T @ (v||1)  +  sk^T @ (retr*(-rkn))
    # so only the second term sits on the cross-segment critical chain.
    zcur = None
    if s < S - 1:
        uc_ps = mem_ps_p.tile([128, DH + 1], F32, tag="mem", name=f"uc_{s}_{hi}")
        for j in range(4):
            nc.tensor.matmul(
                uc_ps[:], sk_nat[:, j, :], v_ones[:, j, :],
                start=(j == 0), stop=(s == 0 and j == 3),
                skip_group_check=True,
            )
    # retr side (the chain): retr = sk @ M; retr_n = retr * (-rkn).
    # Per-pair retr_n tiles keep the uc accumulation's deps exact: the j=0/1
    # matmuls fire as soon as pair 0's stt lands, overlapping pair 1's.
    # high_priority: the cross-segment chain ops should be picked FIRST by
    # the scheduler the moment their deps are ready.
    amem_cat = None
    if 0 < s < S - 1:
        hp = tc.high_priority()
        hp.__enter__()
        retr_ns = []
        for pair in range(2):
            rps = mem_ps_p.tile([128, 2, DH + 1], F32, tag="mem",
                                name=f"retr_{s}_{hi}_{pair}")
            for i2 in range(2):
                nc.tensor.matmul(
                    rps[:, i2, :], sk_bf[:, ts(pair * 2 + i2, 128)], mzb_prev[:],
                    start=(i2 == 0), stop=(i2 == 1), skip_group_check=True,
                )
            # sk.z >= ~e^-1 * 512 after segment 0, so the +EPS is numerically
            # irrelevant: reciprocal straight off the psum z-column, and the
            # negation folds into the scale stt.
            rkn = small.tile([128, 2], F32, tag="rkn", name=f"rkn_{s}_{hi}_{pair}")
            nc.vector.reciprocal(rkn[:], rps[:, :, DH])
            rkn_bc = bass.AP(
                tensor=rkn.tensor, offset=rkn.offset,
                ap=[rkn.ap[0], rkn.ap[1], [0, 128]],
            )
            retr_n = work.tile([128, 2, 128], BF16, tag="retr_n", bufs=6,
                               name=f"retr_n_{s}_{hi}_{pair}")
            nc.vector.scalar_tensor_tensor(
                out=retr_n[:], in0=rps[:, :, :DH],
                scalar=-1.0, in1=rkn_bc, op0=ALU.mult, op1=ALU.mult,
            )
            retr_ns.append(retr_n)
        for j in range(4):
            nc.tensor.matmul(
                uc_ps[:, :DH], sk_nat[:, j, :], retr_ns[j // 2][:, j % 2, :],
                start=False, stop=(j == 3), skip_group_check=True,
            )
        hp.__exit__(None, None, None)
    if s < S - 1:
        if s == 0:
            nc.vector.tensor_copy(mzb_new[:], uc_ps[:])
            nc.vector.tensor_copy(mzf[:], uc_ps[:])
        else:
            with tc.high_priority():
                nc.vector.scalar_tensor_tensor(
                    out=mzb_new[:], in0=uc_ps[:], scalar=1.0, in1=mzf[:],
                    op0=ALU.mult, op1=ALU.add,
                )
            if s < S - 2:  # mzf(S-2) has no reader (S-1 skips the update)
                nc.vector.tensor_add(mzf[:], mzf[:], uc_ps[:])

    # a_mem side (off-chain): amem_cat = gate * (sq @ M) / (sq.z + eps)
    amem_box = [None]

    def emit_amem():
        amem_cat = work.tile([128, 4, 128], F32, tag="amem_cat", bufs=6,
                             name=f"amem_cat_{s}_{hi}")
        for pair in range(2):
            aps = mem_ps_p.tile([128, 2, DH + 1], F32, tag="mem",
                                name=f"amem_{s}_{hi}_{pair}")
            for i2 in range(2):
                nc.tensor.matmul(
                    aps[:, i2, :], sq_bf[:, ts(pair * 2 + i2, 128)], mzb_prev[:],
                    start=(i2 == 0), stop=(i2 == 1), skip_group_check=True,
                )
            rg = small.tile([128, 2], F32, tag="rg", name=f"rg_{s}_{hi}_{pair}")
            nc.vector.reciprocal(rg[:], aps[:, :, DH])
            nc.vector.tensor_scalar_mul(rg[:], rg[:], gate_sb[:, 2 * hi : 2 * hi + 1])
            if s >= S - 2:
                # tail is ACT-heavy: do the scale on DVE in one bcast op
                rg_bc = bass.AP(
                    tensor=rg.tensor, offset=rg.offset,
                    ap=[rg.ap[0], rg.ap[1], [0, 128]],
                )
                nc.vector.tensor_mul(
                    amem_cat[:, 2 * pair : 2 * pair + 2, :],
                    aps[:, :, :DH], rg_bc,
                )
            else:
                for i2 in range(2):
                    nc.scalar.activation(
                        amem_cat[:, pair * 2 + i2, :], aps[:, i2, :DH],
                        AF.Identity, scale=rg[:, i2 : i2 + 1],
                    )
        amem_box[0] = amem_cat

    if s > 0:
        emit_amem()

    # ---------- local causal attention (transposed-scores formulation) ----
    # scoresT_j [m-chunk j, t >= j*128] = k_j^T q; ACT exp writes P^T
    # directly; Pool masks the diagonal block; a_dot accumulates against
    # v||1 so column dh holds the softmax denominator.
    adot_pair = []
    for pair in range(2):
        adot_pair.append(
            adot_ps_p.tile([128, 2, DH + 1], F32, tag="adot",
                           name=f"adot_{s}_{hi}_{pair}")
        )
    for j in range(4):
        t_cols = (4 - j) * 128
        sc = sc_ps_p.tile([128, SEG], F32, tag="scores", name=f"scores_{s}_{hi}_{j}")
        nc.tensor.matmul(
            sc[:, :t_cols], k_bf[:, ts(j, 128)], q_bf[:, j * 128 :],
            start=True, stop=True, skip_group_check=True,
        )
        ptj = work.tile([128, t_cols], BF16, tag=f"pt{j}", bufs=3,
                        name=f"pt{j}_{s}_{hi}")
        nc.scalar.activation(ptj[:], sc[:, :t_cols], AF.Exp, scale=INV_SQRT_D)
        # causal mask on the diagonal block: zero P^T[m, t] where m > t
        # (elementwise on Pool, off both the PE and the DVE scan chain)
        nc.gpsimd.tensor_mul(ptj[:, 0:128], ptj[:, 0:128], maskl[:])
        # the diagonal chunk (i == j) waits on the Pool mask, so consume it
        # LAST in each j-group; start=True goes on the first-emitted write
        # per psum bank (clears has_written bank-wide)
        for i in list(range(j + 1, 4)) + [j]:
            pair, i2 = divmod(i, 2)
            nc.tensor.matmul(
                adot_pair[pair][:, i2, :], ptj[:, ts(i - j, 128)],
                v_ones[:, j, :],
                start=(j == 0 and i in (1, 2)), stop=(j == i),
                skip_group_check=True,
            )

    # ---------- combine ----------
    if s == S - 1:
        amem_cat = amem_box[0]
        # tail: per-region (i2) combine -- adot region (pair,i2) stops at
        # j = 2*pair+i2, so earlier regions' chains complete while PE still
        # works on the later adots, shortening the post-PE serial tail
        for pair in range(2):
            for i2 in range(2):
                rdot = small.tile([128, 1], F32, tag="rdot",
                                  name=f"rdot_{s}_{hi}_{pair}_{i2}")
                nc.vector.reciprocal(rdot[:], adot_pair[pair][:, i2, DH : DH + 1])
                nc.vector.tensor_scalar_mul(
                    rdot[:], rdot[:], gate_sb[:, 2 * hi + 1 : 2 * hi + 2]
                )
                rdot_bc = bass.AP(
                    tensor=rdot.tensor, offset=rdot.offset,
                    ap=[rdot.ap[0], rdot.ap[1], [0, 128]],
                )
                tmp = work.tile([128, 1, 128], F32, tag="a_tmp2",
                                name=f"a_tmp_{s}_{hi}_{pair}_{i2}")
                nc.vector.tensor_mul(
                    tmp[:], adot_pair[pair][:, i2 : i2 + 1, :DH], rdot_bc
                )
                nc.vector.tensor_add(
                    a_sb[:, 2 * pair + i2 : 2 * pair + i2 + 1, :], tmp[:],
                    amem_cat[:, 2 * pair + i2 : 2 * pair + i2 + 1, :],
                )
            if store_cb is not None:
                store_cb(pair)
        return zcur
    amem_cat = amem_box[0]
    for pair in range(2):
        rdot = small.tile([128, 2], F32, tag="rdot", name=f"rdot_{s}_{hi}_{pair}")
        nc.vector.reciprocal(rdot[:], adot_pair[pair][:, :, DH])
        nc.vector.tensor_scalar_mul(
            rdot[:], rdot[:], gate_sb[:, 2 * hi + 1 : 2 * hi + 2]
        )
        rdot_bc = bass.AP(
            tensor=rdot.tensor, offset=rdot.offset,
            ap=[rdot.ap[0], rdot.ap[1], [0, 128]],
        )
        a_slice = a_sb[:, 2 * pair : 2 * pair + 2, :]
        if s > 0:
            tmp = work.tile([128, 2, 128], F32, tag="a_tmp",
                            name=f"a_tmp_{s}_{hi}_{pair}")
            nc.vector.tensor_mul(tmp[:], adot_pair[pair][:, :, :DH], rdot_bc)
            nc.vector.tensor_add(
                a_slice, tmp[:], amem_cat[:, 2 * pair : 2 * pair + 2, :]
            )
        else:
            nc.vector.tensor_mul(a_slice, adot_pair[pair][:, :, :DH], rdot_bc)
        if store_cb is not None:
            store_cb(pair)
    return zcur


_NC_CACHE = {}


def _get_nc(has_bias=False):
    if has_bias not in _NC_CACHE:
        _NC_CACHE[has_bias] = _build_program(has_bias)
    return _NC_CACHE[has_bias]


def _host_consts():
    ident = np.eye(128, dtype=ml_dtypes.bfloat16)
    # masku[m,t] = 1 iff m <= t: keep-mask for the diagonal block of P^T
    maskl = np.triu(np.ones((128, 128), np.float32)).astype(ml_dtypes.bfloat16)
    maskr = (MASK_NEG * np.eye(128, dtype=np.float32)).astype(ml_dtypes.bfloat16)
    return ident, maskl, maskr


def kernel(x, w_q, b_q, w_k, b_k, w_v, b_v, beta, _trace=False):
    global LAST_RESULTS
    x = np.asarray(x, dtype=np.float32)
    w_q = np.asarray(w_q, dtype=np.float32)
    b_q = np.asarray(b_q, dtype=np.float32)
    w_k = np.asarray(w_k, dtype=np.float32)
    b_k = np.asarray(b_k, dtype=np.float32)
    w_v = np.asarray(w_v, dtype=np.float32)
    b_v = np.asarray(b_v, dtype=np.float32)
    beta = np.asarray(beta, dtype=np.float32)

    gate = 1.0 / (1.0 + np.exp(-beta))  # sigmoid, [H]
    ident, maskl, maskr = _host_consts()

    in_maps = []
    for c in range(8):
        b = c // 4
        h0 = (c % 4) * 2
        cols = slice(h0 * DH, (h0 + 2) * DH)
        bias_cols = np.stack(
            [
                b_q[h0 * DH : (h0 + 1) * DH], b_q[(h0 + 1) * DH : (h0 + 2) * DH],
                b_k[h0 * DH : (h0 + 1) * DH], b_k[(h0 + 1) * DH : (h0 + 2) * DH],
                b_v[h0 * DH : (h0 + 1) * DH], b_v[(h0 + 1) * DH : (h0 + 2) * DH],
            ],
            axis=1,
        ).astype(np.float32)  # [128, 6]
        g0, g1 = gate[h0], gate[h0 + 1]
        gates_np = np.tile(
            np.array([g0, 1.0 - g0, g1, 1.0 - g1], np.float32), (128, 1)
        )
        in_maps.append(
            {
                "xT": np.ascontiguousarray(x[b].T).astype(ml_dtypes.bfloat16),
                "wq": np.ascontiguousarray(w_q[:, cols]).astype(ml_dtypes.bfloat16),
                "wk": np.ascontiguousarray(w_k[:, cols]).astype(ml_dtypes.bfloat16),
                "wv": np.ascontiguousarray(w_v[:, cols]).astype(ml_dtypes.bfloat16),
                "biases": np.ascontiguousarray(bias_cols),
                "bvT": np.ascontiguousarray(
                    b_v[cols].reshape(1, 2 * DH)
                ).astype(ml_dtypes.bfloat16),
                "gates": gates_np,
                "ident": ident,
                "maskl": maskl,
                "maskr": maskr,
            }
        )

    has_bias = bool(np.any(b_v))
    nc = _get_nc(has_bias)
    LAST_RESULTS = bass_utils.run_bass_kernel_spmd(
        nc, in_maps, core_ids=list(range(8)), trace=_trace
    )

    out = np.empty((B, T, H * DH), np.float32)
    for c in range(8):
        b = c // 4
        h0 = (c % 4) * 2
        out[b, :, h0 * DH : (h0 + 2) * DH] = LAST_RESULTS.results[c]["out"]
    return out

